# revision 16
# baseline (speedup 1.0000x reference)
"""Trainium2 Bass kernel for nn_Attention_42348377538911.

3D attention: x [2, 128, 16, 16, 16] -> qkv 1x1x1 conv -> 4-head attention
over N=4096 positions (dim_head=32) -> out 1x1x1 conv.

Sharding: 8 cores = 2 batches x 4 heads (one (b, h) pair per core).
Each core computes its head's attention and a tensor-parallel partial of the
output projection; host sums the 4 partials per batch and adds b_out.

Cost-model-driven layout (per core):
  P0    : psum[64, 1024] = w_qkT.T @ x-bf16 (q rows scaled); ACT/DVE evacuate
          q/k to f32r SBUF.  vT = x-chunk.T @ w_v into one [128, 1024] psum,
          single-instruction evacuation into ones-augmented vt_aug bf16.
  simT  : psum[128j, 512i] = k-chunk.T @ q  (f32r, 1 cyc/row), two j-chunks
          per [128, 1024] psum tile.
  exp   : split between ACT (true exp -> bf16) and DVE (Schraudolph:
          i16 = rint(s*128/ln2 + 16250.5), bits reinterpreted as bf16).
  AV    : flipped orientation: psum[128i, 33] += ex-tile.T @ vt_aug
          (bf16, 33 cyc/matmul; col 32 of vt_aug = ones -> softmax denom).
  norm  : DVE reciprocal of denom + broadcast multiply -> out_norm bf16.
  outT  : DMA-transpose [128, 128] (i x (c d) -> (c d) x i).
  y     : psum[128, 128c] = w_oT-rep.T @ outT-chunk; ACT/DVE evacuate to
          bf16; DMA to DRAM.
"""

import sys

import numpy as np

if "/opt/trn_rl_repo" not in sys.path:
    sys.path.insert(0, "/opt/trn_rl_repo")

HEADS = 4
DIM_HEAD = 32
B = 2
C = 128
N = 4096          # 16*16*16 spatial positions
NT = 512          # i-tile width
N_IT = N // NT    # 8 i-tiles

A7 = 128.0 / np.log(2.0)    # Schraudolph scale for bf16 bit pattern
B7 = 16250.5                # calibrated offset (min max-rel-err)

_cached = {}


def _build(nd=8, inter_av=True, schraud=True):
    import concourse.bacc as bacc
    import concourse.tile as tile
    import concourse.mybir as mybir
    from concourse.bass import ts
    from concourse import masks

    f32 = mybir.dt.float32
    f32r = mybir.dt.float32r
    bf16 = mybir.dt.bfloat16
    i16 = mybir.dt.int16
    EXP = mybir.ActivationFunctionType.Exp
    ALU = mybir.AluOpType

    nc = bacc.Bacc("TRN2", target_bir_lowering=False, debug=False, num_devices=nd)
    x_d = nc.dram_tensor("x", [C, N], bf16, kind="ExternalInput").ap()
    wqk_d = nc.dram_tensor("w_qk", [C, 64], bf16, kind="ExternalInput").ap()
    wvt_d = nc.dram_tensor("w_vT", [C, DIM_HEAD], bf16, kind="ExternalInput").ap()
    wor_d = nc.dram_tensor("w_oR", [C, 4 * C], bf16, kind="ExternalInput").ap()
    y_d = nc.dram_tensor("y", [C, N], bf16, kind="ExternalOutput").ap()

    with tile.TileContext(nc) as tc:
        with tc.tile_pool(name="sing", bufs=1) as sing, \
             tc.tile_pool(name="exp", bufs=22) as expp, \
             tc.tile_pool(name="onrm", bufs=2) as onrm, \
             tc.tile_pool(name="odt", bufs=2) as odt, \
             tc.tile_pool(name="ysb", bufs=2) as ysbp, \
             tc.tile_pool(name="rcp", bufs=2) as rcpp:
            wqk = sing.tile([C, 64], bf16)
            wvt = sing.tile([C, DIM_HEAD], bf16)
            wor = sing.tile([C, 4 * C], bf16)
            x_sb = [sing.tile([C, 1024], bf16, tag=f"x{cx}", name=f"x{cx}")
                    for cx in range(4)]
            # q/k in f32r: q_t/k_t[i2] hold columns 1024*i2 .. +1024
            q_t = [sing.tile([32, 1024], f32r, tag=f"q{cx}", name=f"q{cx}")
                   for cx in range(4)]
            k_t = [sing.tile([32, 1024], f32r, tag=f"k{cx}", name=f"k{cx}")
                   for cx in range(4)]
            vt_aug = sing.tile([C, 32, 33], bf16)   # [j, chunk, d + ones]

            # x0 + the two matmul-critical weights go first on HWDGE;
            # the rest ride the gpsimd software DGE (no HWDGE slot).
            nc.sync.dma_start(x_sb[0], x_d[:, 0:1024])
            nc.sync.dma_start(wqk, wqk_d)
            nc.sync.dma_start(wor, wor_d)
            for cx in range(1, 4):
                nc.gpsimd.dma_start(x_sb[cx], x_d[:, 1024 * cx:1024 * (cx + 1)])
            nc.gpsimd.dma_start(wvt, wvt_d)
            # ones for the denominator column (overwritten cols get vT)
            nc.gpsimd.memset(vt_aug[:], 1.0)
            # warm the ACT exp table while DMAs run
            scr = sing.tile([1, 64], f32)
            nc.vector.memset(scr, 0.0)
            nc.scalar.activation(scr, scr, EXP)

            with tc.tile_pool(name="p0qk", bufs=2, space="PSUM") as p0qk, \
                 tc.tile_pool(name="p0v", bufs=1, space="PSUM") as p0v:

                # ---- PE warm-up: junk matmuls on the weight tiles ----
                warm = p0v.tile([C, 1024], f32, tag="pv")
                for _ in range(16):
                    nc.tensor.matmul(warm[:, 0:128], lhsT=wor[:, 0:128],
                                     rhs=wor[:, 0:128], start=True, stop=True)

                # ---------------- P0: projections ----------------
                for i2 in range(4):
                    pqk = p0qk.tile([64, 1024], f32, tag="pqk")
                    for h in range(2):
                        nc.tensor.matmul(pqk[:, ts(h, NT)], lhsT=wqk,
                                         rhs=x_sb[i2][:, ts(h, NT)],
                                         start=True, stop=True)
                    # evacuate q (ACT) and k (DVE) to f32r
                    nc.scalar.copy(q_t[i2][:], pqk[0:32, :].bitcast(f32r))
                    nc.vector.tensor_copy(k_t[i2][:], pqk[32:64, :].bitcast(f32r))

                # vT: one [128, 1024] psum holds all 32 chunks x 32 dims
                pv = p0v.tile([C, 1024], f32, tag="pv")
                for jc in range(32):
                    nc.tensor.matmul(
                        pv[:, ts(jc, 32)],
                        lhsT=x_sb[jc // 8][:, ts(jc % 8, 128)],
                        rhs=wvt, start=True, stop=True)
                nc.scalar.copy(
                    vt_aug[:, :, 0:DIM_HEAD],
                    pv[:].rearrange("p (c d) -> p c d", d=DIM_HEAD))

            with tc.tile_pool(name="simp", bufs=3, space="PSUM") as simp, \
                 tc.tile_pool(name="avp", bufs=1, space="PSUM") as avp, \
                 tc.tile_pool(name="yp", bufs=1, space="PSUM") as yp:

                # ---------------- P1: attention ----------------
                # per i-tile: 16 sim groups of [128, 1024] (2 j-chunks each)
                # exp engine pattern: 9 ACT / 7 DVE per 16 groups
                act_groups = {0, 2, 4, 6, 8, 10, 12, 14, 15}

                ex_tiles = [None] * N_IT
                onr_t = [None] * N_IT
                tail_st = {}
                for step in range(N_IT + 2):
                    do_sim = step < N_IT
                    do_av = 1 <= step <= N_IT
                    do_tail = step >= 2            # for i-tile step-2
                    if do_av:
                        av = avp.tile([C, 132], f32, tag="av")
                        prev = ex_tiles[step - 1]
                    if do_tail:
                        it2 = step - 2
                        dt_t = odt.tile([C, C], bf16, tag="dt")
                        nc.sync.dma_start(dt_t, onr_t[it2], transpose=True)

                    def emit_y_tail():
                        ypt = yp.tile([C, NT], f32, tag="y")
                        # w_oR holds 4 row-masked copies of w_o.T so each
                        # matmul reads all 128 partitions of dt_t (sliced
                        # tile_position reads of one psum bank hang).
                        for c in range(4):
                            nc.tensor.matmul(
                                ypt[:, ts(c, 128)],
                                lhsT=wor[:, ts(c, 128)],
                                rhs=dt_t, start=True, stop=True)
                        y_sb = ysbp.tile([C, NT], bf16, tag="ysb")
                        nc.vector.tensor_copy(y_sb, ypt)
                        nc.sync.dma_start(y_d[:, ts(it2, NT)], y_sb)
                        ex_tiles[it2] = None

                    cur = []
                    for g in range(16):
                        if do_sim:
                            ps = simp.tile([C, 1024], f32, tag="sim")
                            for h in range(2):
                                jc = 2 * g + h
                                nc.tensor.matmul(
                                    ps[:, ts(h, NT)],
                                    lhsT=k_t[jc // 8][:, ts(jc % 8, 128)],
                                    rhs=q_t[step // 2][:, ts(step % 2, NT)],
                                    start=True, stop=True)
                            ex_t = expp.tile([C, 1024], bf16, tag="ex")
                            if g in act_groups or not schraud:
                                nc.scalar.activation(ex_t, ps, EXP)
                            else:
                                nc.vector.tensor_scalar(
                                    ex_t[:].bitcast(i16), ps, A7, B7,
                                    ALU.mult, ALU.add)
                            cur.append(ex_t)

                        # AV matmuls of the previous i-tile, interleaved.
                        # NOTE: each psum region's 32-step accumulation must
                        # run consecutively (c-major) — interleaving regions
                        # within a bank corrupts the accumulation (probe5).
                        if do_av and inter_av:
                            c = g // 4
                            for jj in range(8):
                                jc = 8 * (g % 4) + jj
                                ext = prev[jc // 2]
                                nc.tensor.matmul(
                                    av[:, 33 * c:33 * c + 33],
                                    lhsT=ext[:, ts(4 * (jc % 2) + c, 128)],
                                    rhs=vt_aug[:, jc, :],
                                    start=(jc == 0), stop=(jc == 31))
                        if do_tail and g == 5 and (do_sim or do_av):
                            emit_y_tail()
                    if do_av and not inter_av:
                        for c in range(4):
                            for jc in range(32):
                                ext = prev[jc // 2]
                                nc.tensor.matmul(
                                    av[:, 33 * c:33 * c + 33],
                                    lhsT=ext[:, ts(4 * (jc % 2) + c, 128)],
                                    rhs=vt_aug[:, jc, :],
                                    start=(jc == 0), stop=(jc == 31))
                    if do_sim:
                        ex_tiles[step] = cur
                    if do_tail and not (do_sim or do_av):
                        emit_y_tail()          # final drain phase

                    # end of phase: normalize i-tile step-1 from its av psum
                    if do_av:
                        it = step - 1
                        av3 = av[:].rearrange("p (c w) -> p c w", w=33)
                        rcp = rcpp.tile([C, 4], f32, tag="rcp")
                        nc.vector.reciprocal(rcp, av3[:, :, 32])
                        onr = onrm.tile([C, C], bf16, tag="onr")
                        nc.vector.tensor_tensor(
                            onr[:].rearrange("p (c d) -> p c d", d=DIM_HEAD),
                            av3[:, :, 0:DIM_HEAD],
                            rcp[:, :, None].broadcast_to([C, 4, DIM_HEAD]),
                            ALU.mult)
                        onr_t[it] = onr

    nc.compile()
    return nc


def _get_nc():
    if "nc" not in _cached:
        _cached["nc"] = _build()
    return _cached["nc"]


def _to_bf16(a):
    import ml_dtypes
    return np.asarray(a, dtype=np.float32).astype(ml_dtypes.bfloat16)


def _mask_wo(w_o):
    m = np.zeros((C, 4 * C), dtype=np.float32)
    for c in range(4):
        m[32 * c:32 * c + 32, 128 * c:128 * (c + 1)] = w_o.T
    return m


def _make_in_maps(x, w_qkv, w_out):
    scale = DIM_HEAD ** -0.5
    in_maps = []
    for core in range(8):
        b, h = core // HEADS, core % HEADS
        w_q = w_qkv[h * DIM_HEAD:(h + 1) * DIM_HEAD, :]
        w_k = w_qkv[128 + h * DIM_HEAD:128 + (h + 1) * DIM_HEAD, :]
        w_v = w_qkv[256 + h * DIM_HEAD:256 + (h + 1) * DIM_HEAD, :]
        w_o = w_out[:, h * DIM_HEAD:(h + 1) * DIM_HEAD]  # [128, 32]
        in_maps.append({
            "x": _to_bf16(x[b].reshape(C, N)),
            "w_qk": _to_bf16(
                np.concatenate([w_q.T * scale, w_k.T], axis=1)),
            "w_vT": _to_bf16(w_v.T),
            "w_oR": _to_bf16(_mask_wo(w_o)),
        })
    return in_maps


def _gather(results, b_out):
    y = np.zeros((B, C, N), dtype=np.float32)
    for core in range(8):
        y[core // HEADS] += np.asarray(results[core]["y"], dtype=np.float32)
    y += b_out.astype(np.float32)[None, :, None]
    return y.reshape(B, C, 16, 16, 16)


def run(x, w_qkv, w_out, b_out, trace=False):
    from concourse.bass_utils import run_bass_kernel_spmd
    nc = _get_nc()
    in_maps = _make_in_maps(np.asarray(x), np.asarray(w_qkv), np.asarray(w_out))
    res = run_bass_kernel_spmd(nc, in_maps, core_ids=list(range(8)),
                               trace=trace)
    return _gather(res.results, np.asarray(b_out)), res


def kernel(x, w_qkv, w_out, b_out):
    y, _ = run(x, w_qkv, w_out, b_out)
    return y


# revision 17
# speedup vs baseline: 1.0389x; 1.0389x over previous
"""Trainium2 Bass kernel for nn_Attention_42348377538911.

3D attention: x [2, 128, 16, 16, 16] -> qkv 1x1x1 conv -> 4-head attention
over N=4096 positions (dim_head=32) -> out 1x1x1 conv.

Sharding: 8 cores = 2 batches x 4 heads (one (b, h) pair per core).
Each core computes its head's attention and a tensor-parallel partial of the
output projection; host sums the 4 partials per batch and adds b_out.

Cost-model-driven layout (per core):
  P0    : psum[64, 1024] = w_qkT.T @ x-bf16 (q rows scaled); ACT/DVE evacuate
          q/k to f32r SBUF.  vT = x-chunk.T @ w_v into one [128, 1024] psum,
          single-instruction evacuation into ones-augmented vt_aug bf16.
  simT  : psum[128j, 512i] = k-chunk.T @ q  (f32r, 1 cyc/row), two j-chunks
          per [128, 1024] psum tile.
  exp   : split between ACT (true exp -> bf16) and DVE (Schraudolph:
          i16 = rint(s*128/ln2 + 16250.5), bits reinterpreted as bf16).
  AV    : flipped orientation: psum[128i, 33] += ex-tile.T @ vt_aug
          (bf16, 33 cyc/matmul; col 32 of vt_aug = ones -> softmax denom).
  norm  : DVE reciprocal of denom + broadcast multiply -> out_norm bf16.
  outT  : DMA-transpose [128, 128] (i x (c d) -> (c d) x i).
  y     : psum[128, 128c] = w_oT-rep.T @ outT-chunk; ACT/DVE evacuate to
          bf16; DMA to DRAM.
"""

import sys

import numpy as np

if "/opt/trn_rl_repo" not in sys.path:
    sys.path.insert(0, "/opt/trn_rl_repo")

HEADS = 4
DIM_HEAD = 32
B = 2
C = 128
N = 4096          # 16*16*16 spatial positions
NT = 512          # i-tile width
N_IT = N // NT    # 8 i-tiles

A7 = 128.0 / np.log(2.0)    # Schraudolph scale for bf16 bit pattern
B7 = 16250.5                # calibrated offset (min max-rel-err)

_cached = {}


def _build(nd=8, inter_av=True, schraud=True):
    import concourse.bacc as bacc
    import concourse.tile as tile
    import concourse.mybir as mybir
    from concourse.bass import ts
    from concourse import masks

    f32 = mybir.dt.float32
    f32r = mybir.dt.float32r
    bf16 = mybir.dt.bfloat16
    i16 = mybir.dt.int16
    EXP = mybir.ActivationFunctionType.Exp
    ALU = mybir.AluOpType

    nc = bacc.Bacc("TRN2", target_bir_lowering=False, debug=False, num_devices=nd)
    x_d = nc.dram_tensor("x", [C, N], bf16, kind="ExternalInput").ap()
    wqk_d = nc.dram_tensor("w_qk", [C, 64], bf16, kind="ExternalInput").ap()
    wvt_d = nc.dram_tensor("w_vT", [C, DIM_HEAD], bf16, kind="ExternalInput").ap()
    wor_d = nc.dram_tensor("w_oR", [C, 4 * C], bf16, kind="ExternalInput").ap()
    y_d = nc.dram_tensor("y", [C, N], bf16, kind="ExternalOutput").ap()

    with tile.TileContext(nc) as tc:
        with tc.tile_pool(name="sing", bufs=1) as sing, \
             tc.tile_pool(name="exp", bufs=22) as expp, \
             tc.tile_pool(name="onrm", bufs=2) as onrm, \
             tc.tile_pool(name="odt", bufs=2) as odt, \
             tc.tile_pool(name="ysb", bufs=2) as ysbp, \
             tc.tile_pool(name="rcp", bufs=2) as rcpp:
            wqk = sing.tile([C, 64], bf16)
            wvt = sing.tile([C, DIM_HEAD], bf16)
            wor = sing.tile([C, 4 * C], bf16)
            ident = sing.tile([C, C], bf16)
            x_sb = [sing.tile([C, 1024], bf16, tag=f"x{cx}", name=f"x{cx}")
                    for cx in range(4)]
            # q/k in f32r: q_t/k_t[i2] hold columns 1024*i2 .. +1024
            q_t = [sing.tile([32, 1024], f32r, tag=f"q{cx}", name=f"q{cx}")
                   for cx in range(4)]
            k_t = [sing.tile([32, 1024], f32r, tag=f"k{cx}", name=f"k{cx}")
                   for cx in range(4)]
            vt_aug = sing.tile([C, 32, 33], bf16)   # [j, chunk, d + ones]

            # x + matmul-critical weights on HWDGE; wvt on gpsimd SWDGE
            nc.sync.dma_start(x_sb[0], x_d[:, 0:1024])
            nc.sync.dma_start(wqk, wqk_d)
            nc.sync.dma_start(wor, wor_d)
            for cx in range(1, 4):
                nc.sync.dma_start(x_sb[cx], x_d[:, 1024 * cx:1024 * (cx + 1)])
            nc.gpsimd.dma_start(wvt, wvt_d)
            # ones for the denominator column (overwritten cols get vT)
            nc.gpsimd.memset(vt_aug[:], 1.0)
            masks.make_identity(nc, ident[:])
            # warm the ACT exp table while DMAs run
            scr = sing.tile([1, 64], f32)
            nc.vector.memset(scr, 0.0)
            nc.scalar.activation(scr, scr, EXP)

            with tc.tile_pool(name="simp", bufs=3, space="PSUM") as simp, \
                 tc.tile_pool(name="avp", bufs=1, space="PSUM") as avp, \
                 tc.tile_pool(name="yp", bufs=1, space="PSUM") as yp:

                def emit_qk(i2):
                    pqk = simp.tile([C, 1024], f32, tag="sim")
                    for h in range(2):
                        nc.tensor.matmul(pqk[0:64, ts(h, NT)], lhsT=wqk,
                                         rhs=x_sb[i2][:, ts(h, NT)],
                                         start=True, stop=True)
                    nc.scalar.copy(q_t[i2][:], pqk[0:32, :].bitcast(f32r))
                    nc.vector.tensor_copy(k_t[i2][:], pqk[32:64, :].bitcast(f32r))

                def emit_vt(half):
                    pv = simp.tile([C, 1024], f32, tag="sim")
                    for jj in range(16):
                        jc = 16 * half + jj
                        nc.tensor.matmul(
                            pv[:, ts(jj, 32)],
                            lhsT=x_sb[jc // 8][:, ts(jc % 8, 128)],
                            rhs=wvt, start=True, stop=True)
                    nc.scalar.copy(
                        vt_aug[:, 16 * half:16 * (half + 1), 0:DIM_HEAD],
                        pv[:, 0:NT].rearrange("p (c d) -> p c d", d=DIM_HEAD))

                # ---- PE warm-up junk matmuls (into the av-pool slot) ----
                warm = avp.tile([C, 132], f32, tag="av")
                for _ in range(14):
                    nc.tensor.matmul(warm[:, 0:128], lhsT=wor[:, 0:128],
                                     rhs=wor[:, 0:128], start=True, stop=True)

                # ---------------- P0 + P1 ----------------
                emit_qk(0)
                emit_qk(1)
                emit_vt(0)

                # exp engine pattern: 9 ACT / 7 DVE, runs <= 2 adjacent
                act_groups = {0, 2, 4, 5, 7, 9, 11, 13, 15}

                ex_tiles = [None] * N_IT
                onr_t = [None] * N_IT
                for step in range(N_IT + 2):
                    do_sim = step < N_IT
                    do_av = 1 <= step <= N_IT
                    do_tail = step >= 2            # for i-tile step-2
                    if do_av:
                        av = avp.tile([C, 132], f32, tag="av")
                        prev = ex_tiles[step - 1]
                    if do_tail:
                        it2 = step - 2
                        if it2 < N_IT - 1:
                            dt_t = odt.tile([C, C], bf16, tag="dt")
                            nc.sync.dma_start(dt_t, onr_t[it2], transpose=True)

                    def emit_y_tail():
                        if it2 < N_IT - 1:
                            ypt = yp.tile([C, NT], f32, tag="y")
                            # w_oR holds 4 row-masked copies of w_o.T so
                            # each matmul reads all 128 partitions of dt_t
                            # (sliced tile_position reads of one bank hang)
                            for c in range(4):
                                nc.tensor.matmul(
                                    ypt[:, ts(c, 128)],
                                    lhsT=wor[:, ts(c, 128)],
                                    rhs=dt_t, start=True, stop=True)
                            y_sb = ysbp.tile([C, NT], bf16, tag="ysb")
                            nc.vector.tensor_copy(y_sb, ypt)
                        else:
                            # last i-tile: low-latency PE-transpose tail
                            ytr = yp.tile([C, NT], f32, tag="y")
                            trv = ytr[:].bitcast(bf16)
                            for c in range(4):
                                nc.tensor.transpose(
                                    trv[0:32, ts(c, 128)],
                                    onr_t[it2][:, ts(c, 32)], ident[:])
                            dtl = odt.tile([32, NT], bf16, tag="dtl")
                            nc.scalar.copy(dtl, trv[0:32, 0:NT])
                            ypt = yp.tile([C, NT], f32, tag="y")
                            nc.tensor.matmul(ypt, lhsT=wor[0:32, 0:128],
                                             rhs=dtl, start=True, stop=True)
                            y_sb = ysbp.tile([C, NT], bf16, tag="ysb")
                            nc.scalar.copy(y_sb, ypt)
                        nc.gpsimd.dma_start(y_d[:, ts(it2, NT)], y_sb)
                        ex_tiles[it2] = None

                    cur = []
                    for g in range(16):
                        if do_sim:
                            ps = simp.tile([C, 1024], f32, tag="sim")
                            for h in range(2):
                                jc = 2 * g + h
                                nc.tensor.matmul(
                                    ps[:, ts(h, NT)],
                                    lhsT=k_t[jc // 8][:, ts(jc % 8, 128)],
                                    rhs=q_t[step // 2][:, ts(step % 2, NT)],
                                    start=True, stop=True)
                            ex_t = expp.tile([C, 1024], bf16, tag="ex")
                            if g in act_groups or not schraud:
                                nc.scalar.activation(ex_t, ps, EXP)
                            else:
                                nc.vector.tensor_scalar(
                                    ex_t[:].bitcast(i16), ps, A7, B7,
                                    ALU.mult, ALU.add)
                            cur.append(ex_t)

                        # AV matmuls of the previous i-tile, interleaved.
                        # Each psum region's 32-step accumulation must run
                        # consecutively (c-major): interleaving regions
                        # within a bank corrupts the accumulation (probe5).
                        if do_av and inter_av:
                            c = g // 4
                            for jj in range(8):
                                jc = 8 * (g % 4) + jj
                                ext = prev[jc // 2]
                                nc.tensor.matmul(
                                    av[:, 33 * c:33 * c + 33],
                                    lhsT=ext[:, ts(4 * (jc % 2) + c, 128)],
                                    rhs=vt_aug[:, jc, :],
                                    start=(jc == 0), stop=(jc == 31))
                        if step == 0:
                            if g == 3:
                                emit_qk(2)
                            elif g == 5:
                                emit_vt(1)
                            elif g == 7:
                                emit_qk(3)
                        if do_tail and g == 5 and do_sim:
                            emit_y_tail()
                    if do_av and not inter_av:
                        for c in range(4):
                            for jc in range(32):
                                ext = prev[jc // 2]
                                nc.tensor.matmul(
                                    av[:, 33 * c:33 * c + 33],
                                    lhsT=ext[:, ts(4 * (jc % 2) + c, 128)],
                                    rhs=vt_aug[:, jc, :],
                                    start=(jc == 0), stop=(jc == 31))
                    if do_sim:
                        ex_tiles[step] = cur

                    # end of phase: normalize i-tile step-1 from its av psum
                    if do_av:
                        it = step - 1
                        av3 = av[:].rearrange("p (c w) -> p c w", w=33)
                        rcp = rcpp.tile([C, 4], f32, tag="rcp")
                        nc.vector.reciprocal(rcp, av3[:, :, 32])
                        onr = onrm.tile([C, C], bf16, tag="onr")
                        nc.vector.tensor_tensor(
                            onr[:].rearrange("p (c d) -> p c d", d=DIM_HEAD),
                            av3[:, :, 0:DIM_HEAD],
                            rcp[:, :, None].broadcast_to([C, 4, DIM_HEAD]),
                            ALU.mult)
                        onr_t[it] = onr

                    # tail for phases with no sim groups (steps 8, 9)
                    if do_tail and not do_sim:
                        emit_y_tail()

    nc.compile()
    return nc


def _get_nc():
    if "nc" not in _cached:
        _cached["nc"] = _build()
    return _cached["nc"]


def _to_bf16(a):
    import ml_dtypes
    return np.asarray(a, dtype=np.float32).astype(ml_dtypes.bfloat16)


def _mask_wo(w_o):
    m = np.zeros((C, 4 * C), dtype=np.float32)
    for c in range(4):
        m[32 * c:32 * c + 32, 128 * c:128 * (c + 1)] = w_o.T
    return m


def _make_in_maps(x, w_qkv, w_out):
    scale = DIM_HEAD ** -0.5
    in_maps = []
    for core in range(8):
        b, h = core // HEADS, core % HEADS
        w_q = w_qkv[h * DIM_HEAD:(h + 1) * DIM_HEAD, :]
        w_k = w_qkv[128 + h * DIM_HEAD:128 + (h + 1) * DIM_HEAD, :]
        w_v = w_qkv[256 + h * DIM_HEAD:256 + (h + 1) * DIM_HEAD, :]
        w_o = w_out[:, h * DIM_HEAD:(h + 1) * DIM_HEAD]  # [128, 32]
        in_maps.append({
            "x": _to_bf16(x[b].reshape(C, N)),
            "w_qk": _to_bf16(
                np.concatenate([w_q.T * scale, w_k.T], axis=1)),
            "w_vT": _to_bf16(w_v.T),
            "w_oR": _to_bf16(_mask_wo(w_o)),
        })
    return in_maps


def _gather(results, b_out):
    y = np.zeros((B, C, N), dtype=np.float32)
    for core in range(8):
        y[core // HEADS] += np.asarray(results[core]["y"], dtype=np.float32)
    y += b_out.astype(np.float32)[None, :, None]
    return y.reshape(B, C, 16, 16, 16)


def run(x, w_qkv, w_out, b_out, trace=False):
    from concourse.bass_utils import run_bass_kernel_spmd
    nc = _get_nc()
    in_maps = _make_in_maps(np.asarray(x), np.asarray(w_qkv), np.asarray(w_out))
    res = run_bass_kernel_spmd(nc, in_maps, core_ids=list(range(8)),
                               trace=trace)
    return _gather(res.results, np.asarray(b_out)), res


def kernel(x, w_qkv, w_out, b_out):
    y, _ = run(x, w_qkv, w_out, b_out)
    return y


# revision 18
# speedup vs baseline: 1.0854x; 1.0448x over previous
"""Trainium2 Bass kernel for nn_Attention_42348377538911.

3D attention: x [2, 128, 16, 16, 16] -> qkv 1x1x1 conv -> 4-head attention
over N=4096 positions (dim_head=32) -> out 1x1x1 conv.

Sharding: 8 cores = 2 batches x 4 heads (one (b, h) pair per core).
Each core computes its head's attention and a tensor-parallel partial of the
output projection; host sums the 4 partials per batch and adds b_out.

Cost-model-driven layout (per core):
  P0    : psum[64, 1024] = w_qkT.T @ x-bf16 (q rows scaled); ACT/DVE evacuate
          q/k to f32r SBUF.  vT = x-chunk.T @ w_v into one [128, 1024] psum,
          single-instruction evacuation into ones-augmented vt_aug bf16.
  simT  : psum[128j, 512i] = k-chunk.T @ q  (f32r, 1 cyc/row), two j-chunks
          per [128, 1024] psum tile.
  exp   : split between ACT (true exp -> bf16) and DVE (Schraudolph:
          i16 = rint(s*128/ln2 + 16250.5), bits reinterpreted as bf16).
  AV    : flipped orientation: psum[128i, 33] += ex-tile.T @ vt_aug
          (bf16, 33 cyc/matmul; col 32 of vt_aug = ones -> softmax denom).
  norm  : DVE reciprocal of denom + broadcast multiply -> out_norm bf16.
  outT  : DMA-transpose [128, 128] (i x (c d) -> (c d) x i).
  y     : psum[128, 128c] = w_oT-rep.T @ outT-chunk; ACT/DVE evacuate to
          bf16; DMA to DRAM.
"""

import sys

import numpy as np

if "/opt/trn_rl_repo" not in sys.path:
    sys.path.insert(0, "/opt/trn_rl_repo")

HEADS = 4
DIM_HEAD = 32
B = 2
C = 128
N = 4096          # 16*16*16 spatial positions
NT = 512          # i-tile width
N_IT = N // NT    # 8 i-tiles

A7 = 128.0 / np.log(2.0)    # Schraudolph scale for bf16 bit pattern
B7 = 16250.5                # calibrated offset (min max-rel-err)

_cached = {}


def _build(nd=8, inter_av=True, schraud=True):
    import concourse.bacc as bacc
    import concourse.tile as tile
    import concourse.mybir as mybir
    from concourse.bass import ts
    from concourse import masks

    f32 = mybir.dt.float32
    f32r = mybir.dt.float32r
    bf16 = mybir.dt.bfloat16
    i16 = mybir.dt.int16
    EXP = mybir.ActivationFunctionType.Exp
    ALU = mybir.AluOpType

    nc = bacc.Bacc("TRN2", target_bir_lowering=False, debug=False, num_devices=nd)
    x_d = nc.dram_tensor("x", [C, N], bf16, kind="ExternalInput").ap()
    wqk_d = nc.dram_tensor("w_qk", [C, 64], bf16, kind="ExternalInput").ap()
    wvt_d = nc.dram_tensor("w_vT", [C, DIM_HEAD], bf16, kind="ExternalInput").ap()
    wor_d = nc.dram_tensor("w_oR", [C, 4 * C], bf16, kind="ExternalInput").ap()
    y_d = nc.dram_tensor("y", [C, N], bf16, kind="ExternalOutput").ap()

    with tile.TileContext(nc) as tc:
        with tc.tile_pool(name="sing", bufs=1) as sing, \
             tc.tile_pool(name="exp", bufs=22) as expp, \
             tc.tile_pool(name="onrm", bufs=2) as onrm, \
             tc.tile_pool(name="odt", bufs=2) as odt, \
             tc.tile_pool(name="ysb", bufs=2) as ysbp, \
             tc.tile_pool(name="rcp", bufs=2) as rcpp:
            wqk = sing.tile([C, 64], bf16)
            wvt = sing.tile([C, DIM_HEAD], bf16)
            wor = sing.tile([C, 4 * C], bf16)
            ident = sing.tile([C, C], bf16)
            x_sb = [sing.tile([C, 1024], bf16, tag=f"x{cx}", name=f"x{cx}")
                    for cx in range(4)]
            # q/k in f32r: q_t/k_t[i2] hold columns 1024*i2 .. +1024
            q_t = [sing.tile([32, 1024], f32r, tag=f"q{cx}", name=f"q{cx}")
                   for cx in range(4)]
            k_t = [sing.tile([32, 1024], f32r, tag=f"k{cx}", name=f"k{cx}")
                   for cx in range(4)]
            vt_aug = sing.tile([C, 32, 33], bf16)   # [j, chunk, d + ones]

            # x + matmul-critical weights on HWDGE; wvt on gpsimd SWDGE
            nc.sync.dma_start(x_sb[0], x_d[:, 0:1024])
            nc.sync.dma_start(wqk, wqk_d)
            nc.sync.dma_start(wor, wor_d)
            for cx in range(1, 4):
                nc.sync.dma_start(x_sb[cx], x_d[:, 1024 * cx:1024 * (cx + 1)])
            nc.gpsimd.dma_start(wvt, wvt_d)
            # ones for the denominator column (overwritten cols get vT)
            nc.gpsimd.memset(vt_aug[:], 1.0)
            masks.make_identity(nc, ident[:])
            # warm the ACT exp table while DMAs run
            scr = sing.tile([1, 64], f32)
            nc.vector.memset(scr, 0.0)
            nc.scalar.activation(scr, scr, EXP)

            with tc.tile_pool(name="simp", bufs=3, space="PSUM") as simp, \
                 tc.tile_pool(name="avp", bufs=1, space="PSUM") as avp, \
                 tc.tile_pool(name="yp", bufs=1, space="PSUM") as yp:

                def emit_qk(i2):
                    pqk = simp.tile([C, 1024], f32, tag="sim")
                    for h in range(2):
                        nc.tensor.matmul(pqk[0:64, ts(h, NT)], lhsT=wqk,
                                         rhs=x_sb[i2][:, ts(h, NT)],
                                         start=True, stop=True)
                    nc.scalar.copy(q_t[i2][:], pqk[0:32, :].bitcast(f32r))
                    nc.vector.tensor_copy(k_t[i2][:], pqk[32:64, :].bitcast(f32r))

                def emit_vt(half):
                    pv = simp.tile([C, 1024], f32, tag="sim")
                    for jj in range(16):
                        jc = 16 * half + jj
                        nc.tensor.matmul(
                            pv[:, ts(jj, 32)],
                            lhsT=x_sb[jc // 8][:, ts(jc % 8, 128)],
                            rhs=wvt, start=True, stop=True)
                    nc.scalar.copy(
                        vt_aug[:, 16 * half:16 * (half + 1), 0:DIM_HEAD],
                        pv[:, 0:NT].rearrange("p (c d) -> p c d", d=DIM_HEAD))

                # ---- PE warm-up junk matmuls (into the av-pool slot) ----
                warm = avp.tile([C, 132], f32, tag="av")
                for _ in range(14):
                    nc.tensor.matmul(warm[:, 0:128], lhsT=wor[:, 0:128],
                                     rhs=wor[:, 0:128], start=True, stop=True)

                # ---------------- P0 + P1 ----------------
                emit_qk(0)
                emit_qk(1)
                emit_vt(0)

                # exp engine pattern: 9 ACT / 7 DVE, runs <= 2 adjacent
                act_groups = {0, 2, 4, 5, 7, 9, 11, 13, 15}

                ex_tiles = [None] * N_IT
                onr_t = [None] * N_IT
                for step in range(N_IT + 2):
                    do_sim = step < N_IT
                    do_av = 1 <= step <= N_IT
                    do_tail = step >= 2            # for i-tile step-2
                    if do_av:
                        av = avp.tile([C, 132], f32, tag="av")
                        prev = ex_tiles[step - 1]
                    if do_tail:
                        it2 = step - 2
                        if it2 < N_IT - 1:
                            dt_t = odt.tile([C, C], bf16, tag="dt")
                            nc.sync.dma_start(dt_t, onr_t[it2], transpose=True)

                    def emit_y_tail():
                        if it2 < N_IT - 1:
                            ypt = yp.tile([C, NT], f32, tag="y")
                            # w_oR holds 4 row-masked copies of w_o.T so
                            # each matmul reads all 128 partitions of dt_t
                            # (sliced tile_position reads of one bank hang)
                            for c in range(4):
                                nc.tensor.matmul(
                                    ypt[:, ts(c, 128)],
                                    lhsT=wor[:, ts(c, 128)],
                                    rhs=dt_t, start=True, stop=True)
                            y_sb = ysbp.tile([C, NT], bf16, tag="ysb")
                            nc.vector.tensor_copy(y_sb, ypt)
                        else:
                            # last i-tile: low-latency PE-transpose tail
                            ytr = yp.tile([C, NT], f32, tag="y")
                            trv = ytr[:].bitcast(bf16)
                            for c in range(4):
                                nc.tensor.transpose(
                                    trv[0:32, ts(c, 128)],
                                    onr_t[it2][:, ts(c, 32)], ident[:])
                            dtl = odt.tile([32, NT], bf16, tag="dtl")
                            nc.scalar.copy(dtl, trv[0:32, 0:NT])
                            ypt = yp.tile([C, NT], f32, tag="y")
                            nc.tensor.matmul(ypt, lhsT=wor[0:32, 0:128],
                                             rhs=dtl, start=True, stop=True)
                            y_sb = ysbp.tile([C, NT], bf16, tag="ysb")
                            nc.scalar.copy(y_sb, ypt)
                        nc.gpsimd.dma_start(y_d[:, ts(it2, NT)], y_sb)
                        ex_tiles[it2] = None

                    def emit_norm():
                        it = step - 1
                        av3 = av[:].rearrange("p (c w) -> p c w", w=33)
                        rcp = rcpp.tile([C, 4], f32, tag="rcp")
                        nc.vector.reciprocal(rcp, av3[:, :, 32])
                        onr = onrm.tile([C, C], bf16, tag="onr")
                        nc.vector.tensor_tensor(
                            onr[:].rearrange("p (c d) -> p c d", d=DIM_HEAD),
                            av3[:, :, 0:DIM_HEAD],
                            rcp[:, :, None].broadcast_to([C, 4, DIM_HEAD]),
                            ALU.mult)
                        onr_t[it] = onr

                    cur = []
                    for g in range(16):
                        if do_sim:
                            ps = simp.tile([C, 1024], f32, tag="sim")
                            for h in range(2):
                                jc = 2 * g + h
                                nc.tensor.matmul(
                                    ps[:, ts(h, NT)],
                                    lhsT=k_t[jc // 8][:, ts(jc % 8, 128)],
                                    rhs=q_t[step // 2][:, ts(step % 2, NT)],
                                    start=True, stop=True)
                            ex_t = expp.tile([C, 1024], bf16, tag="ex")
                            if g in act_groups or not schraud:
                                nc.scalar.activation(ex_t, ps, EXP)
                            else:
                                nc.vector.tensor_scalar(
                                    ex_t[:].bitcast(i16), ps, A7, B7,
                                    ALU.mult, ALU.add)
                            cur.append(ex_t)

                        # AV matmuls of the previous i-tile, front-loaded
                        # into the first 8 groups so the accumulator closes
                        # by mid-phase (recip/norm then unblock the next
                        # phase's accumulator).  Each psum region's 32-step
                        # accumulation must run consecutively (c-major):
                        # interleaving regions within a bank corrupts the
                        # accumulation (probe5).
                        if do_av and inter_av and g < 8:
                            c = g // 2
                            for jj in range(16):
                                jc = 16 * (g % 2) + jj
                                ext = prev[jc // 2]
                                nc.tensor.matmul(
                                    av[:, 33 * c:33 * c + 33],
                                    lhsT=ext[:, ts(4 * (jc % 2) + c, 128)],
                                    rhs=vt_aug[:, jc, :],
                                    start=(jc == 0), stop=(jc == 31))
                        if do_av and g == 7:
                            emit_norm()
                        if step == 0:
                            if g == 3:
                                emit_qk(2)
                            elif g == 5:
                                emit_vt(1)
                            elif g == 7:
                                emit_qk(3)
                        if do_tail and g == 5 and do_sim:
                            emit_y_tail()
                    if do_av and not inter_av:
                        for c in range(4):
                            for jc in range(32):
                                ext = prev[jc // 2]
                                nc.tensor.matmul(
                                    av[:, 33 * c:33 * c + 33],
                                    lhsT=ext[:, ts(4 * (jc % 2) + c, 128)],
                                    rhs=vt_aug[:, jc, :],
                                    start=(jc == 0), stop=(jc == 31))
                    if do_sim:
                        ex_tiles[step] = cur

                    # tail for phases with no sim groups (steps 8, 9)
                    if do_tail and not do_sim:
                        emit_y_tail()

    nc.compile()
    return nc


def _get_nc():
    if "nc" not in _cached:
        _cached["nc"] = _build()
    return _cached["nc"]


def _to_bf16(a):
    import ml_dtypes
    return np.asarray(a, dtype=np.float32).astype(ml_dtypes.bfloat16)


def _mask_wo(w_o):
    m = np.zeros((C, 4 * C), dtype=np.float32)
    for c in range(4):
        m[32 * c:32 * c + 32, 128 * c:128 * (c + 1)] = w_o.T
    return m


def _make_in_maps(x, w_qkv, w_out):
    scale = DIM_HEAD ** -0.5
    in_maps = []
    for core in range(8):
        b, h = core // HEADS, core % HEADS
        w_q = w_qkv[h * DIM_HEAD:(h + 1) * DIM_HEAD, :]
        w_k = w_qkv[128 + h * DIM_HEAD:128 + (h + 1) * DIM_HEAD, :]
        w_v = w_qkv[256 + h * DIM_HEAD:256 + (h + 1) * DIM_HEAD, :]
        w_o = w_out[:, h * DIM_HEAD:(h + 1) * DIM_HEAD]  # [128, 32]
        in_maps.append({
            "x": _to_bf16(x[b].reshape(C, N)),
            "w_qk": _to_bf16(
                np.concatenate([w_q.T * scale, w_k.T], axis=1)),
            "w_vT": _to_bf16(w_v.T),
            "w_oR": _to_bf16(_mask_wo(w_o)),
        })
    return in_maps


def _gather(results, b_out):
    y = np.zeros((B, C, N), dtype=np.float32)
    for core in range(8):
        y[core // HEADS] += np.asarray(results[core]["y"], dtype=np.float32)
    y += b_out.astype(np.float32)[None, :, None]
    return y.reshape(B, C, 16, 16, 16)


def run(x, w_qkv, w_out, b_out, trace=False):
    from concourse.bass_utils import run_bass_kernel_spmd
    nc = _get_nc()
    in_maps = _make_in_maps(np.asarray(x), np.asarray(w_qkv), np.asarray(w_out))
    res = run_bass_kernel_spmd(nc, in_maps, core_ids=list(range(8)),
                               trace=trace)
    return _gather(res.results, np.asarray(b_out)), res


def kernel(x, w_qkv, w_out, b_out):
    y, _ = run(x, w_qkv, w_out, b_out)
    return y
